# revision 37
# baseline (speedup 1.0000x reference)
"""Trainium2 Bass kernel for nn_AttributeOperator (MoE-style routing).

Computes out[b] = relu(attr_ops[attrs[b]] @ obj_emb[objs[b]]) for b in [0, B).

Strategy (expert-parallel): the dominant cost is streaming the attr_ops table
(N_ATTRS x D x D fp32 = 512 MB). Samples are grouped by attribute on the host,
groups are load-balanced across the 8 cores (snake deal by group size), and
each core streams only its own subset of operator matrices from HBM exactly
once, quantized on the host to fp8 e3m4 (TRN FP8_EXP3, 4 mantissa bits) with
a global x128 scale folded into the fp16 x vectors — 1 byte/elem halves the
HBM stream vs fp16 (rel err 1.39e-2 vs the f32 reference, under the 2e-2
gate; e4m3 fails at 3.2e-2).

Flipped matmul orientation: the A^T 128x128 chunks are the STATIONARY operand
(fp8 weights trigger the compiler's Fast Weight Load, 4 fp8/partition/cycle),
and the slot's few x columns stream as the moving operand, so PE cost per
matrix is ~16 ldweights at ~27-36 ns instead of 4 N=512 streaming matmuls at
~216 ns (measured 36 us vs 54 us per core). Outputs accumulate TRANSPOSED
(out^T[i, sample]) in 4 [128, ncol] PSUM banks shared by all slots, one ReLU
per bank over all 128 lanes, one contiguous fp16 out^T DMA. Host re-scatters.

Measured on 8 axon trn2 cores: ~64-68 us/iteration, run-to-run ambient
~+/-3 us (fp16 streaming baseline: 108 us; fp8 DMA alone: 47 us; PE path
alone: 39 us — the residual gap is a robust DMA-write vs PE-read
interference, superlinear in PE duty, that no scheduling knob removed:
buffer depth 8-63, 1/2/4-matrix transfers, ring choice/splitting, SWDGE,
engine swaps, act/xt/out splitting all land within noise). ReLU runs on
DVE so the scalar ring owns xt + per-bank output DMAs.
The SPMD program is identical on all 8 cores; only per-core tensors differ.
Slot s has a fixed column capacity maxc[s] = max over cores of that rank's
group size, so the one program fits every core's routing.
"""

import numpy as np
import ml_dtypes

import concourse.tile as tile
from concourse import bacc, mybir
from concourse.bass_utils import run_bass_kernel_spmd

N_CORES = 8
D = 512               # embedding dim (hardcoded per problem spec)
QCH = D // 128        # contraction chunks of 128 partitions
# attr_ops stream is fp8 e3m4 (TRN FP8_EXP3): normals cover [0.25, 15.5], so
# scale A up by 128 (|A|max ~0.11 -> ~13.9) and fold 1/128 into x on the host.
A_SCALE = 128.0
E3M4 = ml_dtypes.float8_e3m4

# test.py hooks (ignored by the grading harness)
LAST_RESULTS = None   # BassKernelResults of the most recent run
TRACE = False
TRACE_CORES = None

PAIR = 1
# Slot column offsets aligned to 4 -> every matmul's f32 PSUM write starts
# 16B-cacheline-aligned and every fp16 xt read 8B-aligned (PSUM lines are
# 8B, SBUF lines 16B; misaligned partial-line PSUM writes measurably slow
# the matmul stream: align=4 beat align=1 by ~5us/iter in-process).
ALIGN = 4
_NC_CACHE = {}


def _build_nc(maxc, offs, ncol, ops_bufs=8, pair=1, sync_frac=(1, 1), reps=1,
              out_engine="scalar", staggered=False, relu_engine="scalar",
              xt_engine="scalar", ops_dt="f8e3", out_dt="f16",
              do_ops_dma=True, do_mm=True, do_act=True, do_out=True):
    """Build + compile the SPMD program.

    maxc[s]: column capacity of slot s; offs[s]: column offset of slot s;
    ncol: total columns (= offs[-1] + maxc[-1]).
    pair: matrices loaded per ops DMA (amortizes per-DMA fixed costs).
    sync_frac: (a, b) -> a of every b ops DMAs issue on sync, rest on scalar.
    reps: hardware-loop repetitions of the whole kernel (for timing).
    staggered: staggered-reset loop back-edge — wedges this device, keep False.
    """
    nm = len(maxc)
    nmp = -(-nm // pair) * pair  # nm rounded up to a multiple of pair
    ng = nmp // pair
    mdt = {"f8e3": mybir.dt.float8e3, "f8e4": mybir.dt.float8e4,
           "f16": mybir.dt.float16}[ops_dt]
    odt = {"f16": mybir.dt.float16, "f32": mybir.dt.float32}[out_dt]
    nc = bacc.Bacc("TRN2", target_bir_lowering=False, debug=False,
                   num_devices=N_CORES)
    # per-group layout [p, t, q, i]: each partition's data is one contiguous
    # pair*QCH*D-element run -> one big DMA descriptor per partition
    ops_dram = nc.dram_tensor("ops_t", [ng, 128, pair * QCH * D],
                              mdt, kind="ExternalInput").ap()
    xt_dram = nc.dram_tensor("xt", [128, QCH * ncol], mybir.dt.float16,
                             kind="ExternalInput").ap()
    out_dram = nc.dram_tensor("out", [ncol, D], odt,
                              kind="ExternalOutput").ap()

    with tile.TileContext(nc) as tc:
        with (
            tc.tile_pool(name="xt", bufs=1) as xt_pool,
            tc.tile_pool(name="ops", bufs=ops_bufs) as ops_pool,
            tc.tile_pool(name="ps", bufs=8, space="PSUM") as ps_pool,
            tc.tile_pool(name="o", bufs=4) as o_pool,
        ):
            def body():
                xt_sb = xt_pool.tile([128, QCH * ncol], mybir.dt.float16)
                getattr(nc, xt_engine).dma_start(xt_sb[:], xt_dram[:])
                if not do_ops_dma:
                    m0 = ops_pool.tile([128, pair * QCH * D], mdt, tag="m")
                    nc.sync.dma_start(m0[:], ops_dram[0])

                for g in range(ng):
                    if do_ops_dma:
                        m = ops_pool.tile([128, pair * QCH * D], mdt, tag="m")
                        issuer = nc.sync if g % sync_frac[1] < sync_frac[0] \
                            else nc.scalar
                        issuer.dma_start(m[:], ops_dram[g])
                    else:
                        m = m0
                    for t in range(pair):
                        s = g * pair + t
                        if s >= nm:
                            break
                        cw = maxc[s]
                        if not do_mm:
                            continue
                        ps = ps_pool.tile([cw, D], mybir.dt.float32, tag="ps")
                        for q in range(QCH):
                            lhsT = xt_sb[:, q * ncol + offs[s]:
                                         q * ncol + offs[s] + cw]
                            rhs = m[:, (t * QCH + q) * D:
                                    (t * QCH + q + 1) * D]
                            nc.tensor.matmul(ps[:], lhsT, rhs,
                                             start=(q == 0),
                                             stop=(q == QCH - 1))
                        if not do_act:
                            continue
                        o = o_pool.tile([cw, D], odt, tag="o")
                        if relu_engine == "vector":
                            nc.vector.tensor_scalar_max(o[:], ps[:], 0.0)
                        else:
                            nc.scalar.activation(
                                o[:], ps[:], mybir.ActivationFunctionType.Relu)
                        if not do_out:
                            continue
                        out_eng = getattr(nc, out_engine)
                        out_eng.dma_start(
                            out_dram[offs[s]:offs[s] + cw, :], o[:])

            if reps == 1:
                body()
            else:
                with tc.For_i(0, reps, 1,
                              hint_engines=(mybir.EngineType.PE,),
                              staggered_reset=staggered):
                    body()

    nc.compile()
    return nc


def _build_nc_flip(maxc, offs, ncol, ops_bufs=8, pair=1, sync_frac=(1, 1),
                   reps=1, out_engine="scalar", staggered=False,
                   relu_engine="vector", xt_engine="scalar", ops_dt="f8e3",
                   out_dt="f16", do_ops_dma=True, do_mm=True, do_act=True,
                   do_out=True, mm_src="real", mm_every=1, dma_split=False,
                   ops_engine=None, act_split=2, xt_split=True,
                   out_per_qi=True):
    """Flipped orientation: A chunks are the stationary operand (fp8 weights
    -> fast weight load), x columns stream as the moving operand.

    Per slot s (one operator matrix A), for each output chunk qi and
    contraction chunk qj: ldweights(A^T[qj,qi] 128x128) + matmul over the
    slot's cw x-columns, accumulating out^T[qi*128:+128, cols(s)] in a PSUM
    tile [128, ncol] shared by all slots. One ReLU per qi over the full
    [128, ncol] bank, one contiguous output DMA of out^T.
    """
    nm = len(maxc)
    nmp = -(-nm // pair) * pair
    ng = nmp // pair
    mdt = {"f8e3": mybir.dt.float8e3, "f8e4": mybir.dt.float8e4,
           "f16": mybir.dt.float16}[ops_dt]
    odt = {"f16": mybir.dt.float16, "f32": mybir.dt.float32}[out_dt]
    nc = bacc.Bacc("TRN2", target_bir_lowering=False, debug=False,
                   num_devices=N_CORES)
    # ops_t[g, p, ((t*QCH+qj)*QCH+qi)*128 + i] = s*A_s[qi*128+i, qj*128+p]
    ops_dram = nc.dram_tensor("ops_t", [ng, 128, pair * QCH * QCH * 128],
                              mdt, kind="ExternalInput").ap()
    xt_dram = nc.dram_tensor("xt", [128, QCH * ncol], mybir.dt.float16,
                             kind="ExternalInput").ap()
    # out^T: out_dram[p, qi*ncol + c] = out[c, qi*128+p]
    out_dram = nc.dram_tensor("out", [128, QCH * ncol], odt,
                              kind="ExternalOutput").ap()

    with tile.TileContext(nc) as tc:
        with (
            tc.tile_pool(name="xt", bufs=1) as xt_pool,
            tc.tile_pool(name="ops", bufs=ops_bufs) as ops_pool,
            tc.tile_pool(name="ps", bufs=8, space="PSUM") as ps_pool,
            tc.tile_pool(name="o", bufs=2) as o_pool,
        ):
            def body():
                xt_sb = xt_pool.tile([128, QCH * ncol], mybir.dt.float16)
                if xt_split:
                    for qj in range(QCH):
                        getattr(nc, xt_engine).dma_start(
                            xt_sb[:, qj * ncol:(qj + 1) * ncol],
                            xt_dram[:, qj * ncol:(qj + 1) * ncol])
                else:
                    getattr(nc, xt_engine).dma_start(xt_sb[:], xt_dram[:])
                ps = [ps_pool.tile([128, ncol], mybir.dt.float32, tag="ps",
                                   name=f"ps{qi}")
                      for qi in range(QCH)]
                if not do_ops_dma or mm_src == "m0":
                    m0 = ops_pool.tile([128, pair * QCH * QCH * 128], mdt,
                                       tag="m0", bufs=1)
                    nc.sync.dma_start(m0[:], ops_dram[0])

                for g in range(ng):
                    if do_ops_dma:
                        m = ops_pool.tile([128, pair * QCH * QCH * 128], mdt,
                                          tag="m")
                        if dma_split:
                            h = pair * QCH * QCH * 128 // 2
                            nc.sync.dma_start(m[:, :h], ops_dram[g][:, :h])
                            nc.scalar.dma_start(m[:, h:], ops_dram[g][:, h:])
                        elif ops_engine is not None:
                            getattr(nc, ops_engine).dma_start(
                                m[:], ops_dram[g])
                        else:
                            issuer = nc.sync \
                                if g % sync_frac[1] < sync_frac[0] \
                                else nc.scalar
                            issuer.dma_start(m[:], ops_dram[g])
                        if mm_src == "m0":
                            m = m0
                    else:
                        m = m0
                    for t in range(pair):
                        s = g * pair + t
                        if s >= nm:
                            break
                        cw = maxc[s]
                        if not do_mm or s % mm_every:
                            continue
                        for qi in range(QCH):
                            for qj in range(QCH):
                                ck = ((t * QCH + qj) * QCH + qi) * 128
                                lhsT = m[:, ck:ck + 128]
                                rhs = xt_sb[:, qj * ncol + offs[s]:
                                            qj * ncol + offs[s] + cw]
                                nc.tensor.matmul(
                                    ps[qi][:, offs[s]:offs[s] + cw],
                                    lhsT, rhs, start=(qj == 0),
                                    stop=(qj == QCH - 1))
                if not do_act:
                    return
                o = o_pool.tile([128, QCH * ncol], odt, tag="o")
                for qi in range(QCH):
                    for h in range(act_split):
                        c0 = ncol * h // act_split
                        c1 = ncol * (h + 1) // act_split
                        dst = o[:, qi * ncol + c0:qi * ncol + c1]
                        src = ps[qi][:, c0:c1]
                        if relu_engine == "vector":
                            nc.vector.tensor_scalar_max(dst, src, 0.0)
                        else:
                            nc.scalar.activation(
                                dst, src, mybir.ActivationFunctionType.Relu)
                    if do_out and out_per_qi:
                        getattr(nc, out_engine).dma_start(
                            out_dram[:, qi * ncol:(qi + 1) * ncol],
                            o[:, qi * ncol:(qi + 1) * ncol])
                if do_out and not out_per_qi:
                    getattr(nc, out_engine).dma_start(out_dram[:], o[:])

            if reps == 1:
                body()
            else:
                with tc.For_i(0, reps, 1,
                              hint_engines=(mybir.EngineType.PE,),
                              staggered_reset=staggered):
                    body()

    nc.compile()
    return nc


def _route(attrs):
    """Group sample indices by attribute, chunk to <=128, snake-balance
    across cores. Returns per-core slot lists of (attr_id, idx_array),
    each list sorted by descending group size."""
    order = np.argsort(attrs, kind="stable")
    sorted_attrs = attrs[order]
    uniq, starts, counts = np.unique(sorted_attrs, return_index=True,
                                     return_counts=True)
    chunks = []
    for a, st, c in zip(uniq, starts, counts):
        idx = order[st:st + c]
        for o in range(0, c, 128):
            chunks.append((int(a), idx[o:o + 128]))
    chunks.sort(key=lambda t: -len(t[1]))
    per_core = [[] for _ in range(N_CORES)]
    for i, ch in enumerate(chunks):
        r, pos = divmod(i, N_CORES)
        k = pos if r % 2 == 0 else N_CORES - 1 - pos
        per_core[k].append(ch)
    return per_core


def _layout(per_core, align=1):
    """Per-slot-rank column capacity/offset shared by all cores.

    align: round capacities up so every slot's column offset is a multiple
    of `align` (align=2 makes f32 PSUM writes 8B-cacheline-aligned).
    """
    nm = max(1, max(len(s) for s in per_core))
    maxc = [1] * nm
    for slots in per_core:
        for s, (_, idx) in enumerate(slots):
            maxc[s] = max(maxc[s], len(idx))
    maxc = [-(-c // align) * align for c in maxc]
    offs = [0] * nm
    for s in range(1, nm):
        offs[s] = offs[s - 1] + maxc[s - 1]
    ncol = offs[-1] + maxc[-1]
    return nm, maxc, offs, ncol


def _prepare(attrs, objs, attr_ops, obj_emb, orient="flip", pair=None,
             align=None):
    """Route + build per-core device input maps."""
    if pair is None:
        pair = PAIR
    if align is None:
        align = ALIGN
    per_core = _route(attrs)
    nm, maxc, offs, ncol = _layout(per_core, align=align)
    nmp = -(-nm // pair) * pair

    rep = obj_emb[objs] * np.float32(1.0 / A_SCALE)  # [B, D], 1/s folded in
    ng = nmp // pair
    in_maps = []
    for k in range(N_CORES):
        slots = per_core[k]
        ops_t = np.zeros((ng, 128, pair, QCH, QCH, 128), E3M4)
        r = np.zeros((ncol, D), np.float32)
        for s, (a, idx) in enumerate(slots):
            g, t = divmod(s, pair)
            at = np.clip(attr_ops[a].T * A_SCALE, -15.5, 15.5).astype(E3M4)
            if orient == "flip":
                # ops_t[g, p, t, qj, qi, i] = s*A[qi*128+i, qj*128+p]
                ops_t[g, :, t] = at.reshape(QCH, 128, QCH, 128).transpose(
                    1, 0, 2, 3)
            else:
                # ops_t[g, p, t, q, i] = s*A[i, q*128+p]
                ops_t[g, :, t] = at.reshape(QCH, 128, D).transpose(
                    1, 0, 2).reshape(128, QCH, QCH, 128)
            r[offs[s]:offs[s] + len(idx)] = rep[idx]
        # xt[p, q*ncol + c] = r[c, q*128 + p]
        xt = np.ascontiguousarray(r.reshape(ncol, QCH, 128).transpose(
            2, 1, 0).astype(np.float16)).reshape(128, -1)
        in_maps.append({"ops_t": ops_t.reshape(ng, 128, pair * QCH * D),
                        "xt": xt})
    return per_core, (nm, tuple(maxc), tuple(offs), ncol), in_maps


ORIENT = "flip"


def kernel(attrs, objs, attr_ops, obj_emb):
    global LAST_RESULTS
    attrs = np.asarray(attrs)
    objs = np.asarray(objs)
    attr_ops = np.asarray(attr_ops, dtype=np.float32)
    obj_emb = np.asarray(obj_emb, dtype=np.float32)
    B = attrs.shape[0]
    d = obj_emb.shape[1]
    assert d == D and attr_ops.shape[1:] == (D, D)

    per_core, (nm, maxc, offs, ncol), in_maps = _prepare(
        attrs, objs, attr_ops, obj_emb, orient=ORIENT)

    nc = _NC_CACHE.get((ORIENT, maxc))
    if nc is None:
        build = _build_nc_flip if ORIENT == "flip" else _build_nc
        nc = _NC_CACHE[(ORIENT, maxc)] = build(maxc, offs, ncol, pair=PAIR)

    res = run_bass_kernel_spmd(nc, in_maps, core_ids=list(range(N_CORES)),
                               trace=TRACE, trace_cores=TRACE_CORES)
    LAST_RESULTS = res

    out = np.zeros((B, d), np.float32)
    for k in range(N_CORES):
        out_k = res.results[k]["out"].astype(np.float32)
        if ORIENT == "flip":
            out_k = out_k.reshape(128, QCH, ncol).transpose(2, 1, 0).reshape(
                ncol, D)
        for s, (a, idx) in enumerate(per_core[k]):
            out[idx] = out_k[offs[s]:offs[s] + len(idx)]
    return out



# revision 40
# speedup vs baseline: 1.3520x; 1.3520x over previous
"""Trainium2 Bass kernel for nn_AttributeOperator (MoE-style routing).

Computes out[b] = relu(attr_ops[attrs[b]] @ obj_emb[objs[b]]) for b in [0, B).

Strategy (expert-parallel): the dominant cost is streaming the attr_ops table
(N_ATTRS x D x D fp32 = 512 MB). Samples are grouped by attribute on the host,
groups are load-balanced across the 8 cores (snake deal by group size), and
each core streams only its own subset of operator matrices from HBM exactly
once, quantized on the host to fp8 e3m4 (TRN FP8_EXP3, 4 mantissa bits) with
a global x128 scale folded into the fp16 x vectors — 1 byte/elem halves the
HBM stream vs fp16 (rel err 1.39e-2 vs the f32 reference, under the 2e-2
gate; e4m3 fails at 3.2e-2).

Flipped matmul orientation: the A^T 128x128 chunks are the STATIONARY operand
(fp8 weights trigger the compiler's Fast Weight Load, 4 fp8/partition/cycle),
and the slot's few x columns stream as the moving operand, so PE cost per
matrix is ~16 ldweights at ~27-36 ns instead of 4 N=512 streaming matmuls at
~216 ns (measured 36 us vs 54 us per core). Outputs accumulate TRANSPOSED
(out^T[i, sample]) in 4 [128, ncol] PSUM banks shared by all slots, one ReLU
per bank over all 128 lanes, one contiguous fp16 out^T DMA. Host re-scatters.

Measured on 8 axon trn2 cores: ~65 us/iteration (best clean reading 64.9;
run-to-run ambient drift +/-3 us). fp16 streaming baseline: 108 us; fp8
DMA alone: 47 us; PE path alone: 39 us — the residual gap is a robust
DMA-write vs PE-read interference, superlinear in PE duty, that no
scheduling knob removed (buffer depth 8-63, 1/2/4-matrix transfers, ring
choice/splitting, SWDGE, engine swaps all land within noise). Two things
that DID matter beyond the fp8/flip rewrite: slot column offsets aligned
to 4 so f32 PSUM writes are 16B-cacheline-aligned and fp16 xt reads
8B-aligned (-5 us in-process), and ReLU on DVE with per-bank output DMAs
so the scalar ring owns xt + outputs while PE/sync stream the matrices.
The SPMD program is identical on all 8 cores; only per-core tensors differ.
Slot s has a fixed column capacity maxc[s] = max over cores of that rank's
group size, so the one program fits every core's routing.
"""

import numpy as np
import ml_dtypes

import concourse.tile as tile
from concourse import bacc, mybir
from concourse.bass_utils import run_bass_kernel_spmd

N_CORES = 8
D = 512               # embedding dim (hardcoded per problem spec)
QCH = D // 128        # contraction chunks of 128 partitions
# attr_ops stream is fp8 e3m4 (TRN FP8_EXP3): normals cover [0.25, 15.5], so
# scale A up by 128 (|A|max ~0.11 -> ~13.9) and fold 1/128 into x on the host.
A_SCALE = 128.0
E3M4 = ml_dtypes.float8_e3m4

# test.py hooks (ignored by the grading harness)
LAST_RESULTS = None   # BassKernelResults of the most recent run
TRACE = False
TRACE_CORES = None

PAIR = 1
# Slot column offsets aligned to 4 -> every matmul's f32 PSUM write starts
# 16B-cacheline-aligned and every fp16 xt read 8B-aligned (PSUM lines are
# 8B, SBUF lines 16B; misaligned partial-line PSUM writes measurably slow
# the matmul stream: align=4 beat align=1 by ~5us/iter in-process).
ALIGN = 4
_NC_CACHE = {}


def _build_nc(maxc, offs, ncol, ops_bufs=8, pair=1, sync_frac=(1, 1), reps=1,
              out_engine="scalar", staggered=False, relu_engine="scalar",
              xt_engine="scalar", ops_dt="f8e3", out_dt="f16",
              do_ops_dma=True, do_mm=True, do_act=True, do_out=True):
    """Build + compile the SPMD program.

    maxc[s]: column capacity of slot s; offs[s]: column offset of slot s;
    ncol: total columns (= offs[-1] + maxc[-1]).
    pair: matrices loaded per ops DMA (amortizes per-DMA fixed costs).
    sync_frac: (a, b) -> a of every b ops DMAs issue on sync, rest on scalar.
    reps: hardware-loop repetitions of the whole kernel (for timing).
    staggered: staggered-reset loop back-edge — wedges this device, keep False.
    """
    nm = len(maxc)
    nmp = -(-nm // pair) * pair  # nm rounded up to a multiple of pair
    ng = nmp // pair
    mdt = {"f8e3": mybir.dt.float8e3, "f8e4": mybir.dt.float8e4,
           "f16": mybir.dt.float16}[ops_dt]
    odt = {"f16": mybir.dt.float16, "f32": mybir.dt.float32}[out_dt]
    nc = bacc.Bacc("TRN2", target_bir_lowering=False, debug=False,
                   num_devices=N_CORES)
    # per-group layout [p, t, q, i]: each partition's data is one contiguous
    # pair*QCH*D-element run -> one big DMA descriptor per partition
    ops_dram = nc.dram_tensor("ops_t", [ng, 128, pair * QCH * D],
                              mdt, kind="ExternalInput").ap()
    xt_dram = nc.dram_tensor("xt", [128, QCH * ncol], mybir.dt.float16,
                             kind="ExternalInput").ap()
    out_dram = nc.dram_tensor("out", [ncol, D], odt,
                              kind="ExternalOutput").ap()

    with tile.TileContext(nc) as tc:
        with (
            tc.tile_pool(name="xt", bufs=1) as xt_pool,
            tc.tile_pool(name="ops", bufs=ops_bufs) as ops_pool,
            tc.tile_pool(name="ps", bufs=8, space="PSUM") as ps_pool,
            tc.tile_pool(name="o", bufs=4) as o_pool,
        ):
            def body():
                xt_sb = xt_pool.tile([128, QCH * ncol], mybir.dt.float16)
                getattr(nc, xt_engine).dma_start(xt_sb[:], xt_dram[:])
                if not do_ops_dma:
                    m0 = ops_pool.tile([128, pair * QCH * D], mdt, tag="m")
                    nc.sync.dma_start(m0[:], ops_dram[0])

                for g in range(ng):
                    if do_ops_dma:
                        m = ops_pool.tile([128, pair * QCH * D], mdt, tag="m")
                        issuer = nc.sync if g % sync_frac[1] < sync_frac[0] \
                            else nc.scalar
                        issuer.dma_start(m[:], ops_dram[g])
                    else:
                        m = m0
                    for t in range(pair):
                        s = g * pair + t
                        if s >= nm:
                            break
                        cw = maxc[s]
                        if not do_mm:
                            continue
                        ps = ps_pool.tile([cw, D], mybir.dt.float32, tag="ps")
                        for q in range(QCH):
                            lhsT = xt_sb[:, q * ncol + offs[s]:
                                         q * ncol + offs[s] + cw]
                            rhs = m[:, (t * QCH + q) * D:
                                    (t * QCH + q + 1) * D]
                            nc.tensor.matmul(ps[:], lhsT, rhs,
                                             start=(q == 0),
                                             stop=(q == QCH - 1))
                        if not do_act:
                            continue
                        o = o_pool.tile([cw, D], odt, tag="o")
                        if relu_engine == "vector":
                            nc.vector.tensor_scalar_max(o[:], ps[:], 0.0)
                        else:
                            nc.scalar.activation(
                                o[:], ps[:], mybir.ActivationFunctionType.Relu)
                        if not do_out:
                            continue
                        out_eng = getattr(nc, out_engine)
                        out_eng.dma_start(
                            out_dram[offs[s]:offs[s] + cw, :], o[:])

            if reps == 1:
                body()
            else:
                with tc.For_i(0, reps, 1,
                              hint_engines=(mybir.EngineType.PE,),
                              staggered_reset=staggered):
                    body()

    nc.compile()
    return nc


def _build_nc_flip(maxc, offs, ncol, ops_bufs=8, pair=1, sync_frac=(1, 1),
                   reps=1, out_engine="scalar", staggered=False,
                   relu_engine="vector", xt_engine="scalar", ops_dt="f8e3",
                   out_dt="f16", do_ops_dma=True, do_mm=True, do_act=True,
                   do_out=True, mm_src="real", mm_every=1, dma_split=False,
                   ops_engine=None, act_split=2, xt_split=True,
                   out_per_qi=True, mm_order="qi"):
    """Flipped orientation: A chunks are the stationary operand (fp8 weights
    -> fast weight load), x columns stream as the moving operand.

    Per slot s (one operator matrix A), for each output chunk qi and
    contraction chunk qj: ldweights(A^T[qj,qi] 128x128) + matmul over the
    slot's cw x-columns, accumulating out^T[qi*128:+128, cols(s)] in a PSUM
    tile [128, ncol] shared by all slots. One ReLU per qi over the full
    [128, ncol] bank, one contiguous output DMA of out^T.
    """
    nm = len(maxc)
    nmp = -(-nm // pair) * pair
    ng = nmp // pair
    mdt = {"f8e3": mybir.dt.float8e3, "f8e4": mybir.dt.float8e4,
           "f16": mybir.dt.float16}[ops_dt]
    odt = {"f16": mybir.dt.float16, "f32": mybir.dt.float32}[out_dt]
    nc = bacc.Bacc("TRN2", target_bir_lowering=False, debug=False,
                   num_devices=N_CORES)
    # ops_t[g, p, ((t*QCH+qj)*QCH+qi)*128 + i] = s*A_s[qi*128+i, qj*128+p]
    ops_dram = nc.dram_tensor("ops_t", [ng, 128, pair * QCH * QCH * 128],
                              mdt, kind="ExternalInput").ap()
    xt_dram = nc.dram_tensor("xt", [128, QCH * ncol], mybir.dt.float16,
                             kind="ExternalInput").ap()
    # out^T: out_dram[p, qi*ncol + c] = out[c, qi*128+p]
    out_dram = nc.dram_tensor("out", [128, QCH * ncol], odt,
                              kind="ExternalOutput").ap()

    with tile.TileContext(nc) as tc:
        with (
            tc.tile_pool(name="xt", bufs=1) as xt_pool,
            tc.tile_pool(name="ops", bufs=ops_bufs) as ops_pool,
            tc.tile_pool(name="ps", bufs=8, space="PSUM") as ps_pool,
            tc.tile_pool(name="o", bufs=2) as o_pool,
        ):
            def body():
                xt_sb = xt_pool.tile([128, QCH * ncol], mybir.dt.float16)
                if xt_split:
                    for qj in range(QCH):
                        getattr(nc, xt_engine).dma_start(
                            xt_sb[:, qj * ncol:(qj + 1) * ncol],
                            xt_dram[:, qj * ncol:(qj + 1) * ncol])
                else:
                    getattr(nc, xt_engine).dma_start(xt_sb[:], xt_dram[:])
                ps = [ps_pool.tile([128, ncol], mybir.dt.float32, tag="ps",
                                   name=f"ps{qi}")
                      for qi in range(QCH)]
                if not do_ops_dma or mm_src == "m0":
                    m0 = ops_pool.tile([128, pair * QCH * QCH * 128], mdt,
                                       tag="m0", bufs=1)
                    nc.sync.dma_start(m0[:], ops_dram[0])

                for g in range(ng):
                    if do_ops_dma:
                        m = ops_pool.tile([128, pair * QCH * QCH * 128], mdt,
                                          tag="m")
                        if dma_split:
                            h = pair * QCH * QCH * 128 // 2
                            nc.sync.dma_start(m[:, :h], ops_dram[g][:, :h])
                            nc.scalar.dma_start(m[:, h:], ops_dram[g][:, h:])
                        elif ops_engine is not None:
                            getattr(nc, ops_engine).dma_start(
                                m[:], ops_dram[g])
                        else:
                            issuer = nc.sync \
                                if g % sync_frac[1] < sync_frac[0] \
                                else nc.scalar
                            issuer.dma_start(m[:], ops_dram[g])
                        if mm_src == "m0":
                            m = m0
                    else:
                        m = m0
                    for t in range(pair):
                        s = g * pair + t
                        if s >= nm:
                            break
                        cw = maxc[s]
                        if not do_mm or s % mm_every:
                            continue
                        order = [(qi, qj) for qi in range(QCH)
                                 for qj in range(QCH)] \
                            if mm_order == "qi" else \
                            [(qi, qj) for qj in range(QCH)
                             for qi in range(QCH)]
                        for qi, qj in order:
                            ck = ((t * QCH + qj) * QCH + qi) * 128
                            lhsT = m[:, ck:ck + 128]
                            rhs = xt_sb[:, qj * ncol + offs[s]:
                                        qj * ncol + offs[s] + cw]
                            nc.tensor.matmul(
                                ps[qi][:, offs[s]:offs[s] + cw],
                                lhsT, rhs, start=(qj == 0),
                                stop=(qj == QCH - 1))
                if not do_act:
                    return
                o = o_pool.tile([128, QCH * ncol], odt, tag="o")
                for qi in range(QCH):
                    for h in range(act_split):
                        c0 = ncol * h // act_split
                        c1 = ncol * (h + 1) // act_split
                        dst = o[:, qi * ncol + c0:qi * ncol + c1]
                        src = ps[qi][:, c0:c1]
                        if relu_engine == "vector":
                            nc.vector.tensor_scalar_max(dst, src, 0.0)
                        else:
                            nc.scalar.activation(
                                dst, src, mybir.ActivationFunctionType.Relu)
                    if do_out and out_per_qi:
                        getattr(nc, out_engine).dma_start(
                            out_dram[:, qi * ncol:(qi + 1) * ncol],
                            o[:, qi * ncol:(qi + 1) * ncol])
                if do_out and not out_per_qi:
                    getattr(nc, out_engine).dma_start(out_dram[:], o[:])

            if reps == 1:
                body()
            else:
                with tc.For_i(0, reps, 1,
                              hint_engines=(mybir.EngineType.PE,),
                              staggered_reset=staggered):
                    body()

    nc.compile()
    return nc


def _route(attrs):
    """Group sample indices by attribute, chunk to <=128, snake-balance
    across cores. Returns per-core slot lists of (attr_id, idx_array),
    each list sorted by descending group size."""
    order = np.argsort(attrs, kind="stable")
    sorted_attrs = attrs[order]
    uniq, starts, counts = np.unique(sorted_attrs, return_index=True,
                                     return_counts=True)
    chunks = []
    for a, st, c in zip(uniq, starts, counts):
        idx = order[st:st + c]
        for o in range(0, c, 128):
            chunks.append((int(a), idx[o:o + 128]))
    chunks.sort(key=lambda t: -len(t[1]))
    per_core = [[] for _ in range(N_CORES)]
    for i, ch in enumerate(chunks):
        r, pos = divmod(i, N_CORES)
        k = pos if r % 2 == 0 else N_CORES - 1 - pos
        per_core[k].append(ch)
    return per_core


def _layout(per_core, align=1):
    """Per-slot-rank column capacity/offset shared by all cores.

    align: round capacities up so every slot's column offset is a multiple
    of `align` (align=2 makes f32 PSUM writes 8B-cacheline-aligned).
    """
    nm = max(1, max(len(s) for s in per_core))
    maxc = [1] * nm
    for slots in per_core:
        for s, (_, idx) in enumerate(slots):
            maxc[s] = max(maxc[s], len(idx))
    maxc = [-(-c // align) * align for c in maxc]
    offs = [0] * nm
    for s in range(1, nm):
        offs[s] = offs[s - 1] + maxc[s - 1]
    ncol = offs[-1] + maxc[-1]
    return nm, maxc, offs, ncol


def _prepare(attrs, objs, attr_ops, obj_emb, orient="flip", pair=None,
             align=None):
    """Route + build per-core device input maps."""
    if pair is None:
        pair = PAIR
    if align is None:
        align = ALIGN
    per_core = _route(attrs)
    nm, maxc, offs, ncol = _layout(per_core, align=align)
    nmp = -(-nm // pair) * pair

    rep = obj_emb[objs] * np.float32(1.0 / A_SCALE)  # [B, D], 1/s folded in
    ng = nmp // pair
    in_maps = []
    for k in range(N_CORES):
        slots = per_core[k]
        ops_t = np.zeros((ng, 128, pair, QCH, QCH, 128), E3M4)
        r = np.zeros((ncol, D), np.float32)
        for s, (a, idx) in enumerate(slots):
            g, t = divmod(s, pair)
            at = np.clip(attr_ops[a].T * A_SCALE, -15.5, 15.5).astype(E3M4)
            if orient == "flip":
                # ops_t[g, p, t, qj, qi, i] = s*A[qi*128+i, qj*128+p]
                ops_t[g, :, t] = at.reshape(QCH, 128, QCH, 128).transpose(
                    1, 0, 2, 3)
            else:
                # ops_t[g, p, t, q, i] = s*A[i, q*128+p]
                ops_t[g, :, t] = at.reshape(QCH, 128, D).transpose(
                    1, 0, 2).reshape(128, QCH, QCH, 128)
            r[offs[s]:offs[s] + len(idx)] = rep[idx]
        # xt[p, q*ncol + c] = r[c, q*128 + p]
        xt = np.ascontiguousarray(r.reshape(ncol, QCH, 128).transpose(
            2, 1, 0).astype(np.float16)).reshape(128, -1)
        in_maps.append({"ops_t": ops_t.reshape(ng, 128, pair * QCH * D),
                        "xt": xt})
    return per_core, (nm, tuple(maxc), tuple(offs), ncol), in_maps


ORIENT = "flip"


def kernel(attrs, objs, attr_ops, obj_emb):
    global LAST_RESULTS
    attrs = np.asarray(attrs)
    objs = np.asarray(objs)
    attr_ops = np.asarray(attr_ops, dtype=np.float32)
    obj_emb = np.asarray(obj_emb, dtype=np.float32)
    B = attrs.shape[0]
    d = obj_emb.shape[1]
    assert d == D and attr_ops.shape[1:] == (D, D)

    per_core, (nm, maxc, offs, ncol), in_maps = _prepare(
        attrs, objs, attr_ops, obj_emb, orient=ORIENT)

    nc = _NC_CACHE.get((ORIENT, maxc))
    if nc is None:
        build = _build_nc_flip if ORIENT == "flip" else _build_nc
        nc = _NC_CACHE[(ORIENT, maxc)] = build(maxc, offs, ncol, pair=PAIR)

    res = run_bass_kernel_spmd(nc, in_maps, core_ids=list(range(N_CORES)),
                               trace=TRACE, trace_cores=TRACE_CORES)
    LAST_RESULTS = res

    out = np.zeros((B, d), np.float32)
    for k in range(N_CORES):
        out_k = res.results[k]["out"].astype(np.float32)
        if ORIENT == "flip":
            out_k = out_k.reshape(128, QCH, ncol).transpose(2, 1, 0).reshape(
                ncol, D)
        for s, (a, idx) in enumerate(per_core[k]):
            out[idx] = out_k[offs[s]:offs[s] + len(idx)]
    return out



# revision 42
# speedup vs baseline: 1.3982x; 1.0342x over previous
"""Trainium2 Bass kernel for nn_AttributeOperator (MoE-style routing).

Computes out[b] = relu(attr_ops[attrs[b]] @ obj_emb[objs[b]]) for b in [0, B).

Strategy (expert-parallel): the dominant cost is streaming the attr_ops table
(N_ATTRS x D x D fp32 = 512 MB). Samples are grouped by attribute on the host,
groups are load-balanced across the 8 cores (snake deal by group size), and
each core streams only its own subset of operator matrices from HBM exactly
once, quantized on the host to fp8 e3m4 (TRN FP8_EXP3, 4 mantissa bits) with
a global x128 scale folded into the fp16 x vectors — 1 byte/elem halves the
HBM stream vs fp16 (rel err 1.39e-2 vs the f32 reference, under the 2e-2
gate; e4m3 fails at 3.2e-2).

Flipped matmul orientation: the A^T 128x128 chunks are the STATIONARY operand
(fp8 weights trigger the compiler's Fast Weight Load, 4 fp8/partition/cycle),
and the slot's few x columns stream as the moving operand, so PE cost per
matrix is ~16 ldweights at ~27-36 ns instead of 4 N=512 streaming matmuls at
~216 ns (measured 36 us vs 54 us per core). Outputs accumulate TRANSPOSED
(out^T[i, sample]) in 4 [128, ncol] PSUM banks shared by all slots, one ReLU
per bank over all 128 lanes, one contiguous fp16 out^T DMA. Host re-scatters.

Measured on 8 axon trn2 cores: ~65 us/iteration (best clean reading 64.9;
run-to-run ambient drift +/-3 us). fp16 streaming baseline: 108 us; fp8
DMA alone: 47 us; PE path alone: 39 us — the residual gap is a robust
DMA-write vs PE-read interference, superlinear in PE duty, that no
scheduling knob removed (buffer depth 8-63, 1/2/4-matrix transfers, ring
choice/splitting, SWDGE, engine swaps all land within noise). Two things
that DID matter beyond the fp8/flip rewrite: slot column offsets aligned
to 4 so f32 PSUM writes are 16B-cacheline-aligned and fp16 xt reads
8B-aligned (-5 us in-process), and ReLU on DVE with per-bank output DMAs
so the scalar ring owns xt + outputs while PE/sync stream the matrices.
The SPMD program is identical on all 8 cores; only per-core tensors differ.
Slot s has a fixed column capacity maxc[s] = max over cores of that rank's
group size, so the one program fits every core's routing.
"""

import numpy as np
import ml_dtypes

import concourse.tile as tile
from concourse import bacc, mybir
from concourse.bass_utils import run_bass_kernel_spmd

N_CORES = 8
D = 512               # embedding dim (hardcoded per problem spec)
QCH = D // 128        # contraction chunks of 128 partitions
# attr_ops stream is fp8 e3m4 (TRN FP8_EXP3): normals cover [0.25, 15.5], so
# scale A up by 128 (|A|max ~0.11 -> ~13.9) and fold 1/128 into x on the host.
A_SCALE = 128.0
E3M4 = ml_dtypes.float8_e3m4

# test.py hooks (ignored by the grading harness)
LAST_RESULTS = None   # BassKernelResults of the most recent run
TRACE = False
TRACE_CORES = None

PAIR = 1
# Slot column offsets aligned to 4 -> every matmul's f32 PSUM write starts
# 16B-cacheline-aligned and every fp16 xt read 8B-aligned (PSUM lines are
# 8B, SBUF lines 16B; misaligned partial-line PSUM writes measurably slow
# the matmul stream: align=4 beat align=1 by ~5us/iter in-process).
ALIGN = 4
_NC_CACHE = {}


def _build_nc(maxc, offs, ncol, ops_bufs=8, pair=1, sync_frac=(1, 1), reps=1,
              out_engine="scalar", staggered=False, relu_engine="scalar",
              xt_engine="scalar", ops_dt="f8e3", out_dt="f16",
              do_ops_dma=True, do_mm=True, do_act=True, do_out=True):
    """Build + compile the SPMD program.

    maxc[s]: column capacity of slot s; offs[s]: column offset of slot s;
    ncol: total columns (= offs[-1] + maxc[-1]).
    pair: matrices loaded per ops DMA (amortizes per-DMA fixed costs).
    sync_frac: (a, b) -> a of every b ops DMAs issue on sync, rest on scalar.
    reps: hardware-loop repetitions of the whole kernel (for timing).
    staggered: staggered-reset loop back-edge — wedges this device, keep False.
    """
    nm = len(maxc)
    nmp = -(-nm // pair) * pair  # nm rounded up to a multiple of pair
    ng = nmp // pair
    mdt = {"f8e3": mybir.dt.float8e3, "f8e4": mybir.dt.float8e4,
           "f16": mybir.dt.float16}[ops_dt]
    odt = {"f16": mybir.dt.float16, "f32": mybir.dt.float32}[out_dt]
    nc = bacc.Bacc("TRN2", target_bir_lowering=False, debug=False,
                   num_devices=N_CORES)
    # per-group layout [p, t, q, i]: each partition's data is one contiguous
    # pair*QCH*D-element run -> one big DMA descriptor per partition
    ops_dram = nc.dram_tensor("ops_t", [ng, 128, pair * QCH * D],
                              mdt, kind="ExternalInput").ap()
    xt_dram = nc.dram_tensor("xt", [128, QCH * ncol], mybir.dt.float16,
                             kind="ExternalInput").ap()
    out_dram = nc.dram_tensor("out", [ncol, D], odt,
                              kind="ExternalOutput").ap()

    with tile.TileContext(nc) as tc:
        with (
            tc.tile_pool(name="xt", bufs=1) as xt_pool,
            tc.tile_pool(name="ops", bufs=ops_bufs) as ops_pool,
            tc.tile_pool(name="ps", bufs=8, space="PSUM") as ps_pool,
            tc.tile_pool(name="o", bufs=4) as o_pool,
        ):
            def body():
                xt_sb = xt_pool.tile([128, QCH * ncol], mybir.dt.float16)
                getattr(nc, xt_engine).dma_start(xt_sb[:], xt_dram[:])
                if not do_ops_dma:
                    m0 = ops_pool.tile([128, pair * QCH * D], mdt, tag="m")
                    nc.sync.dma_start(m0[:], ops_dram[0])

                for g in range(ng):
                    if do_ops_dma:
                        m = ops_pool.tile([128, pair * QCH * D], mdt, tag="m")
                        issuer = nc.sync if g % sync_frac[1] < sync_frac[0] \
                            else nc.scalar
                        issuer.dma_start(m[:], ops_dram[g])
                    else:
                        m = m0
                    for t in range(pair):
                        s = g * pair + t
                        if s >= nm:
                            break
                        cw = maxc[s]
                        if not do_mm:
                            continue
                        ps = ps_pool.tile([cw, D], mybir.dt.float32, tag="ps")
                        for q in range(QCH):
                            lhsT = xt_sb[:, q * ncol + offs[s]:
                                         q * ncol + offs[s] + cw]
                            rhs = m[:, (t * QCH + q) * D:
                                    (t * QCH + q + 1) * D]
                            nc.tensor.matmul(ps[:], lhsT, rhs,
                                             start=(q == 0),
                                             stop=(q == QCH - 1))
                        if not do_act:
                            continue
                        o = o_pool.tile([cw, D], odt, tag="o")
                        if relu_engine == "vector":
                            nc.vector.tensor_scalar_max(o[:], ps[:], 0.0)
                        else:
                            nc.scalar.activation(
                                o[:], ps[:], mybir.ActivationFunctionType.Relu)
                        if not do_out:
                            continue
                        out_eng = getattr(nc, out_engine)
                        out_eng.dma_start(
                            out_dram[offs[s]:offs[s] + cw, :], o[:])

            if reps == 1:
                body()
            else:
                with tc.For_i(0, reps, 1,
                              hint_engines=(mybir.EngineType.PE,),
                              staggered_reset=staggered):
                    body()

    nc.compile()
    return nc


def _build_nc_flip(maxc, offs, ncol, ops_bufs=8, pair=1, sync_frac=(1, 1),
                   reps=1, out_engine="scalar", staggered=False,
                   relu_engine="vector", xt_engine="scalar", ops_dt="f8e3",
                   out_dt="f16", do_ops_dma=True, do_mm=True, do_act=True,
                   do_out=True, mm_src="real", mm_every=1, dma_split=False,
                   ops_engine=None, act_split=2, xt_split=True,
                   out_per_qi=True, mm_order="qi", mm_split=1):
    """Flipped orientation: A chunks are the stationary operand (fp8 weights
    -> fast weight load), x columns stream as the moving operand.

    Per slot s (one operator matrix A), for each output chunk qi and
    contraction chunk qj: ldweights(A^T[qj,qi] 128x128) + matmul over the
    slot's cw x-columns, accumulating out^T[qi*128:+128, cols(s)] in a PSUM
    tile [128, ncol] shared by all slots. One ReLU per qi over the full
    [128, ncol] bank, one contiguous output DMA of out^T.
    """
    nm = len(maxc)
    nmp = -(-nm // pair) * pair
    ng = nmp // pair
    mdt = {"f8e3": mybir.dt.float8e3, "f8e4": mybir.dt.float8e4,
           "f16": mybir.dt.float16}[ops_dt]
    odt = {"f16": mybir.dt.float16, "f32": mybir.dt.float32}[out_dt]
    nc = bacc.Bacc("TRN2", target_bir_lowering=False, debug=False,
                   num_devices=N_CORES)
    # ops_t[g, p, ((t*QCH+qj)*QCH+qi)*128 + i] = s*A_s[qi*128+i, qj*128+p]
    ops_dram = nc.dram_tensor("ops_t", [ng, 128, pair * QCH * QCH * 128],
                              mdt, kind="ExternalInput").ap()
    xt_dram = nc.dram_tensor("xt", [128, QCH * ncol], mybir.dt.float16,
                             kind="ExternalInput").ap()
    # out^T: out_dram[p, qi*ncol + c] = out[c, qi*128+p]
    out_dram = nc.dram_tensor("out", [128, QCH * ncol], odt,
                              kind="ExternalOutput").ap()

    with tile.TileContext(nc) as tc:
        with (
            tc.tile_pool(name="xt", bufs=1) as xt_pool,
            tc.tile_pool(name="ops", bufs=ops_bufs) as ops_pool,
            tc.tile_pool(name="ps", bufs=8, space="PSUM") as ps_pool,
            tc.tile_pool(name="o", bufs=2) as o_pool,
        ):
            def body():
                xt_sb = xt_pool.tile([128, QCH * ncol], mybir.dt.float16)
                if xt_split:
                    for qj in range(QCH):
                        getattr(nc, xt_engine).dma_start(
                            xt_sb[:, qj * ncol:(qj + 1) * ncol],
                            xt_dram[:, qj * ncol:(qj + 1) * ncol])
                else:
                    getattr(nc, xt_engine).dma_start(xt_sb[:], xt_dram[:])
                ps = [ps_pool.tile([128, ncol], mybir.dt.float32, tag="ps",
                                   name=f"ps{qi}")
                      for qi in range(QCH)]
                if not do_ops_dma or mm_src == "m0":
                    m0 = ops_pool.tile([128, pair * QCH * QCH * 128], mdt,
                                       tag="m0", bufs=1)
                    nc.sync.dma_start(m0[:], ops_dram[0])

                for g in range(ng):
                    if do_ops_dma:
                        m = ops_pool.tile([128, pair * QCH * QCH * 128], mdt,
                                          tag="m")
                        if dma_split:
                            h = pair * QCH * QCH * 128 // 2
                            nc.sync.dma_start(m[:, :h], ops_dram[g][:, :h])
                            nc.scalar.dma_start(m[:, h:], ops_dram[g][:, h:])
                        elif ops_engine is not None:
                            getattr(nc, ops_engine).dma_start(
                                m[:], ops_dram[g])
                        else:
                            issuer = nc.sync \
                                if g % sync_frac[1] < sync_frac[0] \
                                else nc.scalar
                            issuer.dma_start(m[:], ops_dram[g])
                        if mm_src == "m0":
                            m = m0
                    else:
                        m = m0
                    for t in range(pair):
                        s = g * pair + t
                        if s >= nm:
                            break
                        cw = maxc[s]
                        if not do_mm or s % mm_every:
                            continue
                        order = [(qi, qj) for qi in range(QCH)
                                 for qj in range(QCH)] \
                            if mm_order == "qi" else \
                            [(qi, qj) for qj in range(QCH)
                             for qi in range(QCH)]
                        for qi, qj in order:
                            ck = ((t * QCH + qj) * QCH + qi) * 128
                            lhsT = m[:, ck:ck + 128]
                            for h in range(mm_split):
                                a0 = offs[s] + cw * h // mm_split
                                a1 = offs[s] + cw * (h + 1) // mm_split
                                if a1 == a0:
                                    continue
                                rhs = xt_sb[:, qj * ncol + a0:
                                            qj * ncol + a1]
                                nc.tensor.matmul(
                                    ps[qi][:, a0:a1],
                                    lhsT, rhs, start=(qj == 0),
                                    stop=(qj == QCH - 1))
                if not do_act:
                    return
                o = o_pool.tile([128, QCH * ncol], odt, tag="o")
                for qi in range(QCH):
                    for h in range(act_split):
                        c0 = ncol * h // act_split
                        c1 = ncol * (h + 1) // act_split
                        dst = o[:, qi * ncol + c0:qi * ncol + c1]
                        src = ps[qi][:, c0:c1]
                        if relu_engine == "vector":
                            nc.vector.tensor_scalar_max(dst, src, 0.0)
                        else:
                            nc.scalar.activation(
                                dst, src, mybir.ActivationFunctionType.Relu)
                    if do_out and out_per_qi:
                        getattr(nc, out_engine).dma_start(
                            out_dram[:, qi * ncol:(qi + 1) * ncol],
                            o[:, qi * ncol:(qi + 1) * ncol])
                if do_out and not out_per_qi:
                    getattr(nc, out_engine).dma_start(out_dram[:], o[:])

            if reps == 1:
                body()
            else:
                with tc.For_i(0, reps, 1,
                              hint_engines=(mybir.EngineType.PE,),
                              staggered_reset=staggered):
                    body()

    nc.compile()
    return nc


def _route(attrs):
    """Group sample indices by attribute, chunk to <=128, snake-balance
    across cores. Returns per-core slot lists of (attr_id, idx_array),
    each list sorted by descending group size."""
    order = np.argsort(attrs, kind="stable")
    sorted_attrs = attrs[order]
    uniq, starts, counts = np.unique(sorted_attrs, return_index=True,
                                     return_counts=True)
    chunks = []
    for a, st, c in zip(uniq, starts, counts):
        idx = order[st:st + c]
        for o in range(0, c, 128):
            chunks.append((int(a), idx[o:o + 128]))
    chunks.sort(key=lambda t: -len(t[1]))
    per_core = [[] for _ in range(N_CORES)]
    for i, ch in enumerate(chunks):
        r, pos = divmod(i, N_CORES)
        k = pos if r % 2 == 0 else N_CORES - 1 - pos
        per_core[k].append(ch)
    return per_core


def _layout(per_core, align=1):
    """Per-slot-rank column capacity/offset shared by all cores.

    align: round capacities up so every slot's column offset is a multiple
    of `align` (align=2 makes f32 PSUM writes 8B-cacheline-aligned).
    """
    nm = max(1, max(len(s) for s in per_core))
    maxc = [1] * nm
    for slots in per_core:
        for s, (_, idx) in enumerate(slots):
            maxc[s] = max(maxc[s], len(idx))
    maxc = [-(-c // align) * align for c in maxc]
    offs = [0] * nm
    for s in range(1, nm):
        offs[s] = offs[s - 1] + maxc[s - 1]
    ncol = offs[-1] + maxc[-1]
    return nm, maxc, offs, ncol


def _prepare(attrs, objs, attr_ops, obj_emb, orient="flip", pair=None,
             align=None):
    """Route + build per-core device input maps."""
    if pair is None:
        pair = PAIR
    if align is None:
        align = ALIGN
    per_core = _route(attrs)
    nm, maxc, offs, ncol = _layout(per_core, align=align)
    nmp = -(-nm // pair) * pair

    rep = obj_emb[objs] * np.float32(1.0 / A_SCALE)  # [B, D], 1/s folded in
    ng = nmp // pair
    in_maps = []
    for k in range(N_CORES):
        slots = per_core[k]
        ops_t = np.zeros((ng, 128, pair, QCH, QCH, 128), E3M4)
        r = np.zeros((ncol, D), np.float32)
        for s, (a, idx) in enumerate(slots):
            g, t = divmod(s, pair)
            at = np.clip(attr_ops[a].T * A_SCALE, -15.5, 15.5).astype(E3M4)
            if orient == "flip":
                # ops_t[g, p, t, qj, qi, i] = s*A[qi*128+i, qj*128+p]
                ops_t[g, :, t] = at.reshape(QCH, 128, QCH, 128).transpose(
                    1, 0, 2, 3)
            else:
                # ops_t[g, p, t, q, i] = s*A[i, q*128+p]
                ops_t[g, :, t] = at.reshape(QCH, 128, D).transpose(
                    1, 0, 2).reshape(128, QCH, QCH, 128)
            r[offs[s]:offs[s] + len(idx)] = rep[idx]
        # xt[p, q*ncol + c] = r[c, q*128 + p]
        xt = np.ascontiguousarray(r.reshape(ncol, QCH, 128).transpose(
            2, 1, 0).astype(np.float16)).reshape(128, -1)
        in_maps.append({"ops_t": ops_t.reshape(ng, 128, pair * QCH * D),
                        "xt": xt})
    return per_core, (nm, tuple(maxc), tuple(offs), ncol), in_maps


ORIENT = "flip"


def kernel(attrs, objs, attr_ops, obj_emb):
    global LAST_RESULTS
    attrs = np.asarray(attrs)
    objs = np.asarray(objs)
    attr_ops = np.asarray(attr_ops, dtype=np.float32)
    obj_emb = np.asarray(obj_emb, dtype=np.float32)
    B = attrs.shape[0]
    d = obj_emb.shape[1]
    assert d == D and attr_ops.shape[1:] == (D, D)

    per_core, (nm, maxc, offs, ncol), in_maps = _prepare(
        attrs, objs, attr_ops, obj_emb, orient=ORIENT)

    nc = _NC_CACHE.get((ORIENT, maxc))
    if nc is None:
        build = _build_nc_flip if ORIENT == "flip" else _build_nc
        nc = _NC_CACHE[(ORIENT, maxc)] = build(maxc, offs, ncol, pair=PAIR)

    res = run_bass_kernel_spmd(nc, in_maps, core_ids=list(range(N_CORES)),
                               trace=TRACE, trace_cores=TRACE_CORES)
    LAST_RESULTS = res

    out = np.zeros((B, d), np.float32)
    for k in range(N_CORES):
        out_k = res.results[k]["out"].astype(np.float32)
        if ORIENT == "flip":
            out_k = out_k.reshape(128, QCH, ncol).transpose(2, 1, 0).reshape(
                ncol, D)
        for s, (a, idx) in enumerate(per_core[k]):
            out[idx] = out_k[offs[s]:offs[s] + len(idx)]
    return out

